# revision 1
# baseline (speedup 1.0000x reference)
"""BitLinear (ternary-quantized linear) Trainium2 kernel.

out = (x @ ternary_quantize(W).T) * mean(|W|),  alpha = 0.7

Sharding: tensor-parallel over out_features (8192 -> 8 x 1024). Every core
gets the full activation x (pre-transposed on host to [K, T] layout so all
device DMAs are contiguous) plus its own weight shard, also pre-transposed
to [K, O_shard].

weight_scale = mean(|W|) needs a global sum of |w| across the 8 shards. A
[128,1] device AllReduce works but costs ~150us per execution in this
environment (vs the ~10us documented floor), so the default is two
launches: launch 1 computes each core's partial |w| sums on device;
the host concatenates the 8 [128,1] vectors (pure data movement, no host
math) and feeds them to every core in launch 2, which sums them on device.
This also removes the weight-scan from launch 2's critical path: the
threshold is available immediately, so quantization pipelines with the
weight DMA and the tensor engine starts ~20us into the kernel.

Device kernel per core in the main launch (SPMD, identical program):
  phase 0: global mean / 0.7*mean threshold replicated on 128 partitions
           (from the gsums input; AllReduce fallback via KERNEL_CC=ar)
  phase 1: ternary-quantize the fp32 shard into fp16 {-1, 0, +1} (exact),
           o-half-major, pipelined with the per-k-tile weight DMAs
  phase 2: tiled matmul, lhsT = x[k,t] fp16 (fp32->fp16 cast in-flight by
           SWDGE DMA), rhs = wq[k,o], accumulate over k=2048 in PSUM
           (16 matmuls of [128,128]x[128,512]), scale by mean(|W|) on the
           scalar engine during the PSUM->SBUF copy, DMA out.

fp16 (not bf16) operands: same 1 cycle/row TensorE throughput, but 10
mantissa bits -> ~2e-4 absmax-relative error instead of ~1.3e-3.
"""

import numpy as np

import concourse.mybir as mybir
import concourse.tile as tile
from concourse import bacc, bass_isa
from concourse.bass_utils import run_bass_kernel_spmd

N_CORES = 8
B, S, IN_F, OUT_F = 4, 2048, 2048, 8192
T_FULL = B * S              # 8192 tokens
K = IN_F                    # contraction dim
OS = OUT_F // N_CORES       # 1024 out-features per core
P = 128
KT = K // P                 # 16 k-tiles
ALPHA = 0.7
N_TOTAL = float(OUT_F * K)  # 2**24, so 1/N_TOTAL is exact in fp32

C = 512                     # token chunk per x DMA
NF = 512                    # matmul moving free dim (one PSUM bank fp32)

LAST_RESULTS = None         # test harness peeks at exec_time_ns here
_PROGRAMS = {}              # compiled program cache across kernel() calls

import os as _os
SKIP = set(filter(None, _os.environ.get("KERNEL_SKIP", "").split(",")))
XDT_NAME = _os.environ.get("KERNEL_XDT", "float16")  # matmul operand dtype
SCALE_ENG = _os.environ.get("KERNEL_SCALE_ENG", "scalar")  # psum-scale engine
CC_KIND = _os.environ.get("KERNEL_CC", "2launch")  # ar | ag | 2launch


def _build_program(t_tokens=T_FULL, loops=1, n_cores=N_CORES, barrier=False):
    F32 = mybir.dt.float32
    AX = mybir.AxisListType.X
    Alu = mybir.AluOpType

    assert t_tokens % C == 0 and C % P == 0 and OS % NF == 0

    nc = bacc.Bacc(
        "TRN2", target_bir_lowering=False, debug=False, num_devices=n_cores
    )
    xT = nc.dram_tensor("xT", [K, t_tokens], F32, kind="ExternalInput").ap()
    wT = nc.dram_tensor("wT", [K, OS], F32, kind="ExternalInput").ap()
    gsums = None
    if CC_KIND == "2launch":
        gsums = nc.dram_tensor(
            "gsums", [P, N_CORES], F32, kind="ExternalInput"
        ).ap()
    out = nc.dram_tensor("out", [t_tokens, OS], F32, kind="ExternalOutput").ap()

    with tile.TileContext(nc) as tc:
        for _loop in range(loops):
            if barrier and _loop > 0:
                tc.strict_bb_all_engine_barrier()
            _build_body(tc, nc, xT, wT, out, t_tokens, n_cores, gsums)

    nc.compile()
    return nc


def _build_phase_a(loops=1, barrier=False):
    """Tiny first launch: per-core sum of |w shard| -> [128, 1] output."""
    F32 = mybir.dt.float32
    AX = mybir.AxisListType.X
    Alu = mybir.AluOpType
    nc = bacc.Bacc(
        "TRN2", target_bir_lowering=False, debug=False, num_devices=N_CORES
    )
    wT = nc.dram_tensor("wT", [K, OS], F32, kind="ExternalInput").ap()
    asum_out = nc.dram_tensor("asum", [P, 1], F32, kind="ExternalOutput").ap()
    with tile.TileContext(nc) as tc:
        for _loop in range(loops):
            if barrier and _loop > 0:
                tc.strict_bb_all_engine_barrier()
            with (
                tc.tile_pool(name="wpa", bufs=3) as wpa,
                tc.tile_pool(name="cpa", bufs=1) as cpa,
            ):
                # 8 x 1MB DMAs (2 k-tiles each): above the DMA batching knee,
                # reduction still pipelines with the loads
                wT_t = wT.rearrange("(n p) o -> p n o", p=P)
                KG = 2
                asum = cpa.tile([P, KT // KG], F32)
                for g in range(KT // KG):
                    wt = wpa.tile([P, KG, OS], F32, name="wt", tag="wt")
                    nc.sync.dma_start(wt[:], wT_t[:, g * KG : (g + 1) * KG, :])
                    nc.vector.tensor_reduce(
                        asum[:, g : g + 1], wt[:],
                        axis=mybir.AxisListType.XY, op=Alu.add,
                        apply_absolute_value=True,
                    )
                asum1 = cpa.tile([P, 1], F32)
                nc.vector.tensor_reduce(asum1[:], asum[:], axis=AX, op=Alu.add)
                nc.sync.dma_start(asum_out[:], asum1[:])
    nc.compile()
    return nc


def _build_phase_a_raw(loops=1):
    """Raw-bass phase A: no TileContext, so no ~10-15us exit butterfly.

    8 HWDGE DMAs into distinct SBUF regions (no reuse -> no WAR hazards),
    DVE abs-reduces pipelined behind them via one counting semaphore.
    """
    F32 = mybir.dt.float32
    Alu = mybir.AluOpType
    XY = mybir.AxisListType.XY
    AX = mybir.AxisListType.X
    KG = 2
    NG = KT // KG  # 8 groups
    nc = bacc.Bacc(
        "TRN2", target_bir_lowering=False, debug=False, num_devices=N_CORES
    )
    wT = nc.dram_tensor("wT", [K, OS], F32, kind="ExternalInput").ap()
    asum_out = nc.dram_tensor("asum", [P, 1], F32, kind="ExternalOutput").ap()
    wT_t = wT.rearrange("(n p) o -> p n o", p=P)

    import contextlib

    with contextlib.ExitStack() as ctx:
        wt = ctx.enter_context(nc.sbuf_tensor("wt", [P, KT, OS], F32))
        asum_sb = ctx.enter_context(nc.sbuf_tensor("asum_sb", [P, NG], F32))
        asum1_sb = ctx.enter_context(nc.sbuf_tensor("asum1_sb", [P, 1], F32))
        # one completion sem per DMA group: HWDGE DMAs complete out of
        # order, so a single counting sem cannot gate per-group reduces
        gsem = [
            ctx.enter_context(nc.semaphore(f"dma_sem{g}")) for g in range(NG)
        ]
        osem = ctx.enter_context(nc.semaphore("out_sem"))
        vec_sem = ctx.enter_context(nc.semaphore("vec_sem"))
        block = ctx.enter_context(nc.Block())

        @block.sync
        def _(sync):
            for it in range(loops):
                for g in range(NG):
                    sync.dma_start(
                        wt.ap()[:, g * KG : (g + 1) * KG, :],
                        wT_t[:, g * KG : (g + 1) * KG, :],
                    ).then_inc(gsem[g], 16)
                sync.wait_ge(vec_sem, it + 1)
                sync.dma_start(asum_out[:], asum1_sb.ap()[:]).then_inc(
                    osem, 16
                )
                # last byte of the output must land before the stream ends
                sync.wait_ge(osem, (it + 1) * 16)

        @block.vector
        def _(vector):
            for it in range(loops):
                for g in range(NG):
                    vector.wait_ge(gsem[g], (it + 1) * 16)
                    vector.tensor_reduce(
                        asum_sb.ap()[:, g : g + 1],
                        wt.ap()[:, g * KG : (g + 1) * KG, :],
                        axis=XY, op=Alu.add, apply_absolute_value=True,
                    )
                vector.tensor_reduce(
                    asum1_sb.ap()[:], asum_sb.ap()[:], axis=AX, op=Alu.add
                ).then_inc(vec_sem, 1)

    nc.compile()
    return nc


def _build_body(tc, nc, xT, wT, out, t_tokens, n_cores, gsums=None):
    F32 = mybir.dt.float32
    XDT = getattr(mybir.dt, XDT_NAME)
    AX = mybir.AxisListType.X
    Alu = mybir.AluOpType
    if True:
        with (  # noqa: SIM117

            tc.tile_pool(name="wpool", bufs=1) as wpool,
            tc.tile_pool(name="cpool", bufs=1) as cpool,
            tc.tile_pool(name="dram", bufs=1, space="DRAM") as dram,
            tc.tile_pool(name="xpool", bufs=2) as xpool,
            tc.tile_pool(name="opool", bufs=6) as opool,
            tc.tile_pool(name="psum", bufs=6, space="PSUM") as psum_pool,
        ):
            # ---- phase 0: weight shard load + global mean(|W|) ----
            # per-(o-half, k-tile) DMAs in quantization order, so the first
            # o-half's quant (which gates the first matmuls) only waits for
            # half the weight bytes
            wf = wpool.tile([P, KT, OS], F32)
            wT_t = wT.rearrange("(n p) o -> p n o", p=P)
            for oc in range(OS // NF):
                osl = slice(oc * NF, (oc + 1) * NF)
                for k in range(KT):
                    nc.sync.dma_start(wf[:, k, osl], wT_t[:, k, osl])
            if not (CC_KIND == "2launch" and gsums is not None):
                # local |w| sums feed the collective (non-2launch modes)
                asum = cpool.tile([P, KT], F32)
                for k in range(KT):
                    nc.vector.tensor_reduce(
                        asum[:, k : k + 1], wf[:, k, :], axis=AX, op=Alu.add,
                        apply_absolute_value=True,
                    )
                asum1 = cpool.tile([P, 1], F32)
                nc.vector.tensor_reduce(asum1[:], asum[:], axis=AX, op=Alu.add)

            if CC_KIND == "2launch" and gsums is not None:
                # partial |w| sums of all 8 cores arrive as an input
                gsum8 = cpool.tile([P, N_CORES], F32)
                nc.sync.dma_start(gsum8[:], gsums[:])
                gsum = cpool.tile([P, 1], F32)
                nc.vector.tensor_reduce(gsum[:], gsum8[:], axis=AX, op=Alu.add)
            elif n_cores > 1 and "ar" not in SKIP:
                if CC_KIND == "ag":
                    cc_in = dram.tile([P, 1], F32)
                    cc_out = dram.tile([n_cores * P, 1], F32)
                    nc.sync.dma_start(cc_in[:], asum1[:])
                    nc.gpsimd.collective_compute(
                        "AllGather", Alu.bypass,
                        replica_groups=[list(range(n_cores))],
                        ins=[cc_in.opt()], outs=[cc_out.opt()],
                    )
                    gsum8 = cpool.tile([P, n_cores], F32)
                    nc.sync.dma_start(
                        gsum8[:], cc_out.rearrange("(r p) o -> p (r o)", p=P)
                    )
                    gsum = cpool.tile([P, 1], F32)
                    nc.vector.tensor_reduce(
                        gsum[:], gsum8[:], axis=AX, op=Alu.add
                    )
                else:
                    cc_in = dram.tile([P, 1], F32)
                    cc_out = dram.tile([P, 1], F32)
                    nc.sync.dma_start(cc_in[:], asum1[:])
                    nc.gpsimd.collective_compute(
                        "AllReduce", Alu.add,
                        replica_groups=[list(range(n_cores))],
                        ins=[cc_in.opt()], outs=[cc_out.opt()],
                    )
                    gsum = cpool.tile([P, 1], F32)
                    nc.sync.dma_start(gsum[:], cc_out[:])
            else:
                gsum = asum1  # single-core (TimelineSim) variant

            tot = cpool.tile([P, 1], F32)
            if "par" not in SKIP:
                nc.gpsimd.partition_all_reduce(
                    tot[:], gsum[:], channels=P, reduce_op=bass_isa.ReduceOp.add
                )
            else:
                nc.vector.tensor_copy(tot[:], gsum[:])
            # mean = tot * 2**-24 (exact); thr = 0.7 * mean; both replicated
            mean_t = cpool.tile([P, 1], F32)
            nc.vector.tensor_scalar_mul(mean_t[:], tot[:], 1.0 / N_TOTAL)
            thr_t = cpool.tile([P, 1], F32)
            nc.vector.tensor_scalar_mul(thr_t[:], mean_t[:], ALPHA)
            nthr_t = cpool.tile([P, 1], F32)
            nc.vector.tensor_scalar_mul(nthr_t[:], thr_t[:], -1.0)

            # ---- phase 1: ternary quantize -> wq in bf16 (exact values) ----
            # oc-major so the first o-half is ready in half the quant time;
            # phase 2's first chunk consumes o-half 0 first.
            wq = wpool.tile([P, KT, OS], XDT)
            if "quant" in SKIP:
                nc.vector.memset(wq[:], 1.0)
            else:
                for oc in range(OS // NF):
                    osl = slice(oc * NF, (oc + 1) * NF)
                    for k in range(KT):
                        neg = wpool.tile([P, NF], XDT, tag="negtmp")
                        # neg = (w <= -thr) in {0,1}
                        nc.vector.tensor_scalar(
                            neg[:], wf[:, k, osl], nthr_t[:], None, op0=Alu.is_le
                        )
                        # wq = (w >= thr) - neg  in {-1, 0, 1}
                        nc.vector.scalar_tensor_tensor(
                            wq[:, k, osl], wf[:, k, osl], thr_t[:], neg[:],
                            op0=Alu.is_ge, op1=Alu.subtract,
                        )

            # ---- phase 2: matmul sweep over tokens (oc-major per chunk so
            # the first chunk only waits on the o-half-0 quantization) ----
            xT_t = xT.rearrange("(n p) t -> p n t", p=P)
            n_chunks = t_tokens // C
            for tch in range(n_chunks):
                xb = xpool.tile([P, KT, C], XDT)
                # SWDGE DMA with in-flight fp32 -> bf16 cast
                nc.gpsimd.dma_start(
                    xb[:], xT_t[:, :, tch * C : (tch + 1) * C]
                )
                for oc in range(OS // NF):
                    osl = slice(oc * NF, (oc + 1) * NF)
                    for tsub in range(C // P):
                        t0 = tch * C + tsub * P
                        po = psum_pool.tile([P, NF], F32, name="po", tag="po")
                        if "mm" not in SKIP:
                            for k in range(KT):
                                nc.tensor.matmul(
                                    po[:],
                                    xb[:, k, tsub * P : (tsub + 1) * P],
                                    wq[:, k, osl],
                                    start=(k == 0),
                                    stop=(k == KT - 1),
                                )
                        else:
                            nc.vector.memset(po[:], 0.0)
                        ob = opool.tile([P, NF], F32, name="ob", tag="ob")
                        if "scale" in SKIP:
                            nc.vector.tensor_copy(ob[:], po[:])
                        elif SCALE_ENG == "vector":
                            nc.vector.tensor_scalar_mul(ob[:], po[:], mean_t[:])
                        else:
                            # out = psum * mean(|W|), on the scalar engine
                            nc.scalar.mul(ob[:], po[:], mean_t[:])
                        if "outdma" not in SKIP:
                            nc.sync.dma_start(out[t0 : t0 + P, osl], ob[:])


def kernel(x, weight):
    global LAST_RESULTS
    x = np.asarray(x, dtype=np.float32)
    weight = np.asarray(weight, dtype=np.float32)
    assert x.shape == (B, S, IN_F), x.shape
    assert weight.shape == (OUT_F, IN_F), weight.shape

    xT = np.ascontiguousarray(x.reshape(T_FULL, K).T)
    in_maps = []
    for c in range(N_CORES):
        wTc = np.ascontiguousarray(weight[c * OS : (c + 1) * OS, :].T)
        in_maps.append({"xT": xT, "wT": wTc})

    cores = list(range(N_CORES))
    if CC_KIND == "2launch":
        # launch 1: per-core partial |w| sums (all math on device)
        if "a" not in _PROGRAMS:
            _PROGRAMS["a"] = _build_phase_a()
        res_a = run_bass_kernel_spmd(_PROGRAMS["a"], in_maps, cores)
        gs = np.concatenate(  # pure data movement, no host math
            [res_a.results[c]["asum"] for c in range(N_CORES)], axis=1
        )
        for m in in_maps:
            m["gsums"] = gs
    if "main" not in _PROGRAMS:
        _PROGRAMS["main"] = _build_program()
    res = run_bass_kernel_spmd(_PROGRAMS["main"], in_maps, cores)
    LAST_RESULTS = res
    outs = [res.results[c]["out"] for c in range(N_CORES)]
    return np.concatenate(outs, axis=1).reshape(B, S, OUT_F)



# revision 2
# speedup vs baseline: 365.3575x; 365.3575x over previous
"""BitLinear Trainium2 kernel: fp8 e4m3 DoubleRow matmul.

out = (x @ ternary_quantize(W).T) * mean(|W|),  alpha = 0.7

Tensor-parallel over out_features (8192 -> 8 x 1024), same 2-launch
weight-scale scheme as v1 (phase A partial |w| sums -> host concat ->
launch 2 reduces on device; reduction structure kept IDENTICAL to v1 so
the device fp32 mean lands on the same value — the nearest |w| sits
0.4 ulp from the 0.7*mean threshold for this seed, so the mean must not
move even 1 ulp or a ternary weight flips vs the reference).

Main-launch changes vs v1:
  - Weights ternary-quantized into fp8 e4m3 ({-1,0,+1} exact).
  - x cast fp32 -> fp8 e4m3 in-flight by SWDGE DMA (deterministic RNE;
    quantization noise measured 1.93e-2 absmax-rel on this seed).
  - Matmul in DoubleRow perf mode: 256-deep contraction per pass,
    stationary = wq [128, 2, 128o], moving = x8 [128, 2, C tokens],
    PSUM tile [128o, C]. 2x PE throughput vs bf16/fp16.
  - Output written [o, t]-major as fp16 (halves out DMA); host
    transposes/concats/upcasts (pure data movement + lossless cast).
"""

import numpy as np

import concourse.mybir as mybir
import concourse.tile as tile
from concourse import bacc, bass_isa
from concourse.bass_utils import run_bass_kernel_spmd

N_CORES = 8
B, S, IN_F, OUT_F = 4, 2048, 2048, 8192
T_FULL = B * S              # 8192 tokens
K = IN_F                    # contraction dim
OS = OUT_F // N_CORES       # 1024 out-features per core
P = 128
KT = K // P                 # 16 k-slices of 128
KP = KT // 2                # 8 k-pairs of 256 (DoubleRow)
ALPHA = 0.7
N_TOTAL = float(OUT_F * K)  # 2**24, exact reciprocal in fp32

C = 512                     # token chunk = PSUM bank width (fp32)
OB = 128                    # out-feature block = stationary columns

LAST_RESULTS = None
_PROGRAMS = {}

import os as _os
SKIP = set(filter(None, _os.environ.get("KERNEL_SKIP", "").split(",")))
G = int(_os.environ.get("KERNEL_G", "1"))  # token chunks sharing stationary
# E = number of k-slices (of 16) computed exactly via hi/lo two-plane DR:
# plane0 = e4m3(x), plane1 = e4m3(8*(x - e4m3(x))) against (wq, wq/8).
# Each exact slice costs one extra DR matmul per psum tile (+1/8 PE time
# per 2 slices) and removes its share of e4m3 noise:
# rel_err ~= 1.93e-2 * sqrt((16-E)/16).
E = int(_os.environ.get("KERNEL_E", "0"))
assert E % 2 == 0 and 0 <= E <= KT
OUT32 = bool(int(_os.environ.get("KERNEL_OUT32", "0")))
USE_DR = bool(int(_os.environ.get("KERNEL_DR", "1")))  # 0: plain fp8 MMs
WQDT_NAME = _os.environ.get("KERNEL_WQDT", "float8e4")  # quant target dtype
DR = mybir.MatmulPerfMode.DoubleRow


def _build_phase_a(loops=1, barrier=False):
    """Per-core sum of |w shard| -> [128, 1]. Identical to v1."""
    F32 = mybir.dt.float32
    AX = mybir.AxisListType.X
    Alu = mybir.AluOpType
    nc = bacc.Bacc(
        "TRN2", target_bir_lowering=False, debug=False, num_devices=N_CORES
    )
    wT = nc.dram_tensor("wT", [K, OS], F32, kind="ExternalInput").ap()
    asum_out = nc.dram_tensor("asum", [P, 1], F32, kind="ExternalOutput").ap()
    with tile.TileContext(nc) as tc:
        for _loop in range(loops):
            if barrier and _loop > 0:
                tc.strict_bb_all_engine_barrier()
            with (
                tc.tile_pool(name="wpa", bufs=3) as wpa,
                tc.tile_pool(name="cpa", bufs=1) as cpa,
            ):
                wT_t = wT.rearrange("(n p) o -> p n o", p=P)
                KG = 2
                asum = cpa.tile([P, KT // KG], F32)
                for g in range(KT // KG):
                    wt = wpa.tile([P, KG, OS], F32, name="wt", tag="wt")
                    nc.sync.dma_start(wt[:], wT_t[:, g * KG : (g + 1) * KG, :])
                    nc.vector.tensor_reduce(
                        asum[:, g : g + 1], wt[:],
                        axis=mybir.AxisListType.XY, op=Alu.add,
                        apply_absolute_value=True,
                    )
                asum1 = cpa.tile([P, 1], F32)
                nc.vector.tensor_reduce(asum1[:], asum[:], axis=AX, op=Alu.add)
                nc.sync.dma_start(asum_out[:], asum1[:])
    nc.compile()
    return nc


def _build_main(t_tokens=T_FULL, loops=1, n_cores=N_CORES, barrier=False):
    F32 = mybir.dt.float32
    nc = bacc.Bacc(
        "TRN2", target_bir_lowering=False, debug=False, num_devices=n_cores
    )
    xT = nc.dram_tensor("xT", [K, t_tokens], F32, kind="ExternalInput").ap()
    wT = nc.dram_tensor("wT", [K, OS], F32, kind="ExternalInput").ap()
    gsums = nc.dram_tensor("gsums", [P, N_CORES], F32, kind="ExternalInput").ap()
    outT = nc.dram_tensor(
        "outT", [OS, t_tokens],
        mybir.dt.float32 if OUT32 else mybir.dt.float16,
        kind="ExternalOutput",
    ).ap()

    with tile.TileContext(nc) as tc:
        for _loop in range(loops):
            if barrier and _loop > 0:
                tc.strict_bb_all_engine_barrier()
            _build_body(tc, nc, xT, wT, gsums, outT, t_tokens)

    nc.compile()
    return nc


def _build_body(tc, nc, xT, wT, gsums, outT, t_tokens):
    F32 = mybir.dt.float32
    F16 = mybir.dt.float16
    F8 = getattr(mybir.dt, WQDT_NAME)
    AX = mybir.AxisListType.X
    Alu = mybir.AluOpType

    with (  # noqa: SIM117
        tc.tile_pool(name="wfp", bufs=2) as wfp,
        tc.tile_pool(name="wqp", bufs=1) as wqp,
        tc.tile_pool(name="cpool", bufs=1) as cpool,
        tc.tile_pool(name="xpool", bufs=3) as xpool,
        tc.tile_pool(name="opool", bufs=6) as opool,
        tc.tile_pool(name="psum", bufs=6, space="PSUM") as psum_pool,
    ):
        # ---- phase 0: mean/threshold from gsums (v1 structure, bit-exact)
        gsum8 = cpool.tile([P, N_CORES], F32)
        nc.sync.dma_start(gsum8[:], gsums[:])
        gsum = cpool.tile([P, 1], F32)
        nc.vector.tensor_reduce(gsum[:], gsum8[:], axis=AX, op=Alu.add)
        tot = cpool.tile([P, 1], F32)
        nc.gpsimd.partition_all_reduce(
            tot[:], gsum[:], channels=P, reduce_op=bass_isa.ReduceOp.add
        )
        mean_t = cpool.tile([P, 1], F32)
        nc.vector.tensor_scalar_mul(mean_t[:], tot[:], 1.0 / N_TOTAL)
        thr_t = cpool.tile([P, 1], F32)
        nc.vector.tensor_scalar_mul(thr_t[:], mean_t[:], ALPHA)
        nthr_t = cpool.tile([P, 1], F32)
        nc.vector.tensor_scalar_mul(nthr_t[:], thr_t[:], -1.0)

        # ---- phase 1: weight load + ternary quantize into fp8 e4m3 ----
        # oc-major halves so the first o-blocks unblock the PE early.
        # The last E k-slices go to wqx with (wq, wq/8) plane pairs.
        KS_N = KT - E  # slices handled by normal 2-slice DR pairs
        wT_t = wT.rearrange("(n p) o -> p n o", p=P)  # [128, 16, 1024]
        wq = wqp.tile([P, max(KS_N, 1), OS], F8)
        wqx = wqp.tile([P, E, 2, OS], F8) if E else None
        if "quant" in SKIP:
            nc.vector.memset(wq[:], 1.0)
            if E:
                nc.vector.memset(wqx[:], 1.0)
        else:
            for oc in range(2):
                osl = slice(oc * 512, (oc + 1) * 512)
                wf = wfp.tile([P, KT, 512], F32, name="wf", tag="wf")
                for ks in range(KT):
                    nc.sync.dma_start(wf[:, ks, :], wT_t[:, ks, osl])
                for ks in range(KT):
                    neg = wfp.tile([P, 512], F8, tag="neg")
                    nc.vector.tensor_scalar(
                        neg[:], wf[:, ks, :], nthr_t[:], None, op0=Alu.is_le
                    )
                    tgt = (
                        wq[:, ks, osl] if ks < KS_N
                        else wqx[:, ks - KS_N, 0, osl]
                    )
                    nc.vector.scalar_tensor_tensor(
                        tgt, wf[:, ks, :], thr_t[:], neg[:],
                        op0=Alu.is_ge, op1=Alu.subtract,
                    )
                    if ks >= KS_N:
                        nc.vector.tensor_scalar_mul(
                            wqx[:, ks - KS_N, 1, osl], tgt, 0.125
                        )

        # ---- phase 2: DoubleRow matmul sweep ----
        KP_N = KS_N // 2  # normal k-pairs
        ODT = F32 if OUT32 else F16
        xT_t = xT.rearrange("(n p) t -> p n t", p=P)  # [128, 16, T]
        n_groups = t_tokens // (C * G)
        for tg in range(n_groups):
            x8s, xlo8s = [], []
            for g in range(G):
                tch = tg * G + g
                tsl = slice(tch * C, (tch + 1) * C)
                x8 = xpool.tile([P, KT, C], F8, name="x8", tag="x8")
                if "xdma" in SKIP:
                    nc.vector.memset(x8[:], 1.0)
                else:
                    nc.gpsimd.dma_start(x8[:], xT_t[:, :, tsl])
                x8s.append(x8)
                if E:
                    # exact slices: hi plane copied, lo = e4m3(8*(x - hi))
                    xf32 = xpool.tile([P, E, C], F32, tag="xf32")
                    nc.sync.dma_start(xf32[:], xT_t[:, KS_N:, tsl])
                    xlo8 = xpool.tile([P, E, 2, C], F8, tag="xlo8")
                    for e in range(E):
                        ks = KS_N + e
                        nc.vector.tensor_copy(
                            xlo8[:, e, 0, :], x8[:, ks, :]
                        )
                        hi8 = xpool.tile([P, C], F8, tag="hi8")
                        nc.vector.tensor_scalar_mul(
                            hi8[:], x8[:, ks, :], 8.0
                        )
                        nc.vector.scalar_tensor_tensor(
                            xlo8[:, e, 1, :], xf32[:, e, :], 8.0, hi8[:],
                            op0=Alu.mult, op1=Alu.subtract,
                        )
                    xlo8s.append(xlo8)
            for ob in range(OS // OB):
                obsl = slice(ob * OB, (ob + 1) * OB)
                pos = [
                    psum_pool.tile([P, C], F32, name="po", tag=f"po{g}")
                    for g in range(G)
                ]
                if "mm" in SKIP:
                    for g in range(G):
                        nc.vector.memset(pos[g][:], 0.0)
                elif not USE_DR:
                    for ks in range(KT):
                        for g in range(G):
                            nc.tensor.matmul(
                                pos[g][:],
                                wq[:, ks, obsl],
                                x8s[g][:, ks, :],
                                start=(ks == 0),
                                stop=(ks == KT - 1),
                            )
                else:
                    for kp in range(KP_N):
                        for g in range(G):
                            nc.tensor.matmul(
                                pos[g][:],
                                wq[:, 2 * kp : 2 * kp + 2, obsl],
                                x8s[g][:, 2 * kp : 2 * kp + 2, :],
                                start=(kp == 0),
                                stop=(kp == KP_N - 1 and E == 0),
                                perf_mode=DR,
                            )
                    for e in range(E):
                        for g in range(G):
                            nc.tensor.matmul(
                                pos[g][:],
                                wqx[:, e, :, obsl],
                                xlo8s[g][:, e, :, :],
                                start=(KP_N == 0 and e == 0),
                                stop=(e == E - 1),
                                perf_mode=DR,
                            )
                for g in range(G):
                    tch = tg * G + g
                    ob_t = opool.tile([P, C], ODT, name="ob", tag="ob")
                    nc.scalar.mul(ob_t[:], pos[g][:], mean_t[:])
                    if "outdma" not in SKIP:
                        nc.sync.dma_start(
                            outT[obsl, tch * C : (tch + 1) * C], ob_t[:]
                        )


def kernel(x, weight):
    global LAST_RESULTS
    x = np.asarray(x, dtype=np.float32)
    weight = np.asarray(weight, dtype=np.float32)
    assert x.shape == (B, S, IN_F), x.shape
    assert weight.shape == (OUT_F, IN_F), weight.shape

    xT = np.ascontiguousarray(x.reshape(T_FULL, K).T)
    in_maps = []
    for c in range(N_CORES):
        wTc = np.ascontiguousarray(weight[c * OS : (c + 1) * OS, :].T)
        in_maps.append({"xT": xT, "wT": wTc})

    cores = list(range(N_CORES))
    if "a" not in _PROGRAMS:
        _PROGRAMS["a"] = _build_phase_a()
    res_a = run_bass_kernel_spmd(_PROGRAMS["a"], in_maps, cores)
    gs = np.concatenate(  # pure data movement, no host math
        [res_a.results[c]["asum"] for c in range(N_CORES)], axis=1
    )
    for m in in_maps:
        m["gsums"] = gs

    if "main" not in _PROGRAMS:
        _PROGRAMS["main"] = _build_main()
    res = run_bass_kernel_spmd(_PROGRAMS["main"], in_maps, cores)
    LAST_RESULTS = res
    outs = [res.results[c]["outT"] for c in range(N_CORES)]  # [OS, T] f16
    full = np.concatenate(outs, axis=0)  # [OUT_F, T]
    return np.ascontiguousarray(full.T).astype(np.float32).reshape(
        B, S, OUT_F
    )
